# revision 6
# baseline (speedup 1.0000x reference)
"""BitLinear forward (fake-quant int8 activations x ternary weight) on 8 TRN2 cores.

Data-parallel over the flattened (B*S) token dim: 8192 rows per core, the
[1024,1024] ternary weight replicated per core as fp8e4m3 (exact: values in
{-1,0,1}).

Per-core kernel (per 128-row tile, software-pipelined with LAG=2):
  SP    : x tile in (fp32, 512KB), out tile (bf16, 256KB)
  Pool  : t = x*(1/s) + M1 (fp32, magic round)    [GPSIMD tensor_scalar]
          q = t - M1 -> bf16 (exact ints in [-127,127]; no clamp needed:
          |x*(1/s)| <= 127.0001 rounds to <= 127)
  PE    : 8x 128x128 transpose of q (bf16, raw mode) -> qT in PSUM
  ACT   : h8T = Copy(qT) -> fp8e4m3.  The fp8 RNE cast IS the coarse split:
          for |q| in (2^k, 2^(k+1)], k>=4, it rounds to the 16-level grid, so
          l = q - h8 always fits fp8 exactly (|l| <= 4).
  DVE   : lT = qT - h8T -> fp8e4m3
  PE    : 16 DoubleRow fp8 matmuls: psum[s,o] += sum_k (h8T_k.T @ w_k
          + lT_k.T @ w_k), both planes streaming the same w_k via a
          stride-0 broadcast AP.  DoubleRow = 0.5 cycles/output-column.
  ACT/DVE: epilogue halves: out = psum * scale -> bf16
Host: upcast bf16 -> fp32 and add bias (exact fp32 add; total error is the
bf16 output rounding, ~3e-3 relative, plus quantize boundary flips).

Cost model (steady state per tile): PE 2131ns, DMA 2184ns, Pool 1706ns,
ACT ~1650ns, DVE ~1845ns -> DMA/PE-bound, ~160us per core pass vs 256us
for the bf16-matmul baseline.
"""

import numpy as np
import ml_dtypes

B, S, D = 16, 4096, 1024
N_CORES = 8
ROWS = (B * S) // N_CORES  # 8192 rows per core
P = 128
NT = ROWS // P             # 64 row tiles per core
KT = D // P                # 8 contraction tiles
QB = 127.0
M1 = float(1.5 * 2 ** 23)  # fp32 round-to-nearest-even magic constant
F8 = ml_dtypes.float8_e4m3

_NC_CACHE = {}


def _build_nc(nt=NT, lag=2, xin_bufs=5, work_bufs=5, hl_bufs=5, out_bufs=4,
              pt_bufs=3, po_bufs=2, wt_chunks=4):
    import concourse.mybir as mybir
    from concourse import bacc
    from concourse.tile import TileContext
    from concourse.masks import make_identity

    fp32 = mybir.dt.float32
    bf16 = mybir.dt.bfloat16
    fp8 = mybir.dt.float8e4
    Alu = mybir.AluOpType
    Act = mybir.ActivationFunctionType

    nc = bacc.Bacc(None, target_bir_lowering=False)
    rows = nt * P
    x = nc.dram_tensor("x", [rows, D], fp32, kind="ExternalInput")
    # wt[p, k*2D + pl*D + o] = w[k*128+p, o] for both planes pl in {0,1}
    # (DoubleRow rhs planes materialized with real strides; w = tw.T - 1)
    wt = nc.dram_tensor("wt", [P, KT * 2 * D], fp8, kind="ExternalInput")
    scal = nc.dram_tensor("scal", [P, 2], fp32, kind="ExternalInput")  # [scale, 1/scale]
    out = nc.dram_tensor("out", [rows, D], bf16, kind="ExternalOutput")

    with TileContext(nc) as tc:
        with (
            tc.tile_pool(name="const", bufs=1) as constp,
            tc.tile_pool(name="xin", bufs=xin_bufs) as xp,
            tc.tile_pool(name="work", bufs=work_bufs) as wp,
            tc.tile_pool(name="hlp", bufs=hl_bufs) as hlp,
            tc.tile_pool(name="ptp", bufs=pt_bufs, space="PSUM") as ptp,
            tc.tile_pool(name="pop", bufs=po_bufs, space="PSUM") as pop,
            tc.tile_pool(name="oout", bufs=out_bufs) as op_,
        ):
            ident = constp.tile([P, P], bf16)
            make_identity(nc, ident)
            sc = constp.tile([P, 2], fp32)
            nc.gpsimd.dma_start(out=sc, in_=scal[:, :])
            wt_sb = constp.tile([P, KT * 2 * D], fp8)

            state = {}

            def front(st):
                xa = xp.tile([P, D], fp32, name="xa")
                nc.sync.dma_start(out=xa, in_=x[st * P:(st + 1) * P, :])
                t = wp.tile([P, D], fp32, name="t")
                nc.gpsimd.tensor_scalar(t, xa, sc[:, 1:2], M1, Alu.mult, Alu.add)
                q = wp.tile([P, D], bf16, name="q")
                nc.gpsimd.tensor_scalar(q, t, -M1, None, Alu.add)
                qT = ptp.tile([P, D], bf16, name="qT")
                for k in range(KT):
                    nc.tensor.transpose(
                        qT[:, k * P:(k + 1) * P], q[:, k * P:(k + 1) * P], ident)
                hl = hlp.tile([P, 2 * D], fp8, name="hl")
                nc.scalar.activation(hl[:, 0:D], qT, Act.Copy)
                nc.vector.tensor_tensor(out=hl[:, D:2 * D], in0=qT,
                                        in1=hl[:, 0:D], op=Alu.subtract)
                state[st] = hl

            def back(st, last=False):
                hl = state.pop(st)
                po = pop.tile([P, D], fp32, name="po")
                hl3 = hl[:, :].rearrange("p (two x) -> p two x", two=2)
                for h in range(2):
                    os_ = slice(h * 512, (h + 1) * 512)
                    for k in range(KT):
                        lhs = hl3[:, :, k * P:(k + 1) * P]
                        rhs = wt_sb[:, k * 2 * D:(k + 1) * 2 * D].rearrange(
                            "p (two n) -> p two n", two=2)[:, :, h * 512:(h + 1) * 512]
                        nc.tensor.matmul(
                            po[:, os_], lhs, rhs,
                            start=(k == 0), stop=(k == KT - 1),
                            perf_mode=mybir.MatmulPerfMode.DoubleRow,
                        )
                # epilogue halves on ACT and DVE; store on the ACT ring so
                # the SP ring carries only the input stream
                oo = op_.tile([P, D], bf16, name="oo")
                nc.scalar.activation(oo[:, 0:512], po[:, 0:512], Act.Copy,
                                     scale=sc[:, 0:1])
                nc.vector.tensor_scalar(oo[:, 512:1024], po[:, 512:1024],
                                        sc[:, 0:1], None, Alu.mult)
                eng = nc.sync if st % 2 == 0 else nc.scalar
                eng.dma_start(out=out[st * P:(st + 1) * P, :], in_=oo)

            # weight DMA in chunks on the scalar ring, interleaved between
            # the first two tiles' emissions (all chunks land before back(0))
            wt_cols = KT * 2 * D // wt_chunks
            for st in range(nt):
                front(st)
                if st < 2:
                    for c in range(st * wt_chunks // 2, (st + 1) * wt_chunks // 2):
                        cs = slice(c * wt_cols, (c + 1) * wt_cols)
                        nc.scalar.dma_start(out=wt_sb[:, cs], in_=wt[:, cs])
                if st >= lag:
                    back(st - lag)
            for st in range(max(nt - lag, 0), nt):
                back(st, last=(st == nt - 1))
    nc.compile()
    return nc


def _get_nc(nt=NT):
    if nt not in _NC_CACHE:
        _NC_CACHE[nt] = _build_nc(nt)
    return _NC_CACHE[nt]


def _prep_inputs(x, ternary_weight, bias, act_scale, n_cores=N_CORES, rows=ROWS):
    x = np.asarray(x, dtype=np.float32)
    tw = np.asarray(ternary_weight)

    scale = np.maximum(np.float32(act_scale), np.float32(1e-5))
    inv = np.float32(1.0) / scale

    # w.T [i, o] = tw[o, i] - 1 in {-1,0,1}, exact in fp8e4m3; fold to
    # [128, KT*D]: wt[p, k*D + o] = w[k*128+p, o]
    w = (tw.T.astype(np.float32) - 1.0).astype(F8)  # [D_IN, D_OUT]
    wk = w.reshape(KT, P, D).transpose(1, 0, 2)  # [P, KT, D]
    wt_folded = np.ascontiguousarray(
        np.stack([wk, wk], axis=2).reshape(P, KT * 2 * D))
    scal = np.ascontiguousarray(
        np.broadcast_to(np.array([scale, inv], dtype=np.float32)[None, :], (P, 2)))

    xf = x.reshape(-1, D)
    in_maps = []
    for c in range(n_cores):
        in_maps.append({
            "x": np.ascontiguousarray(xf[c * rows:(c + 1) * rows]),
            "wt": wt_folded,
            "scal": scal,
        })
    return in_maps


def kernel(x, ternary_weight, bias, act_scale):
    from concourse.bass_utils import run_bass_kernel_spmd

    in_maps = _prep_inputs(x, ternary_weight, bias, act_scale)
    nc = _get_nc()
    res = run_bass_kernel_spmd(nc, in_maps, core_ids=list(range(N_CORES)))
    out = np.concatenate([np.asarray(r["out"]) for r in res.results], axis=0)
    out = out.astype(np.float32) + np.asarray(bias, dtype=np.float32)[None, :]
    return out.reshape(B, S, D)


# revision 9
# speedup vs baseline: 1.0143x; 1.0143x over previous
"""BitLinear forward (fake-quant int8 activations x ternary weight) on 8 TRN2 cores.

Data-parallel over the flattened (B*S) token dim: 8192 rows per core, the
[1024,1024] ternary weight replicated per core as fp8e4m3 (exact: values in
{-1,0,1}).

Per-core kernel (per 128-row tile, software-pipelined with LAG tiles between
the front and back halves):
  SP/ACT: x tile in (fp32, 512KB) as two halves, one per HWDGE ring; the
          bf16 out tile alternates rings.  Rings transfer concurrently.
  Pool  : t = x*(1/s) + M1 (fp32, magic round)    [GPSIMD tensor_scalar]
          q = t - M1 -> bf16 (exact ints in [-127,127]; no clamp needed:
          |x*(1/s)| <= 127.0001 rounds to <= 127)
  PE    : 8x 128x128 transpose of q (bf16, raw mode) -> qT in PSUM
  ACT   : h8T = Copy(qT) -> fp8e4m3.  The fp8 RNE cast IS the coarse split:
          for |q| in (2^k, 2^(k+1)], k>=4, it rounds onto a 16-level grid, so
          l = q - h8 always fits fp8 exactly (|l| <= 4).
  DVE   : lT = qT - h8T -> fp8e4m3
  PE    : 16 DoubleRow fp8 matmuls: psum[s,o] += sum_k (h8T_k.T @ w_k
          + lT_k.T @ w_k), both planes streaming the same w_k via a
          stride-0 broadcast AP.  DoubleRow = 0.5 cycles/output-column.
  ACT/DVE: epilogue halves: out = psum * scale -> bf16, each emitted right
          behind its half's matmuls.
Host: upcast bf16 -> fp32 and add bias (exact fp32 add; total error is the
bf16 output rounding, ~3.4e-3 relative, plus quantize boundary flips).

Cost model (steady state per tile): PE 2131ns (bound), Pool 1706ns, DVE
~1845ns, ACT ~1650ns, DMA rings ~1150ns each -> ~140us per core pass vs
256us for the bf16-matmul baseline.
"""

import numpy as np
import ml_dtypes

B, S, D = 16, 4096, 1024
N_CORES = 8
ROWS = (B * S) // N_CORES  # 8192 rows per core
P = 128
NT = ROWS // P             # 64 row tiles per core
KT = D // P                # 8 contraction tiles
QB = 127.0
M1 = float(1.5 * 2 ** 23)  # fp32 round-to-nearest-even magic constant
F8 = ml_dtypes.float8_e4m3

_NC_CACHE = {}


def _build_nc(nt=NT, lag=3, xin_bufs=5, work_bufs=5, hl_bufs=5, out_bufs=4,
              pt_bufs=4, po_bufs=4, wt_chunks=4, fine_tiles=4):
    import concourse.mybir as mybir
    from concourse import bacc
    from concourse.tile import TileContext
    from concourse.masks import make_identity

    fp32 = mybir.dt.float32
    bf16 = mybir.dt.bfloat16
    fp8 = mybir.dt.float8e4
    Alu = mybir.AluOpType
    Act = mybir.ActivationFunctionType

    nc = bacc.Bacc(None, target_bir_lowering=False)
    rows = nt * P
    x = nc.dram_tensor("x", [rows, D], fp32, kind="ExternalInput")
    # wt[p, k*D + o] = w[k*128+p, o], w = ternary_weight.T - 1, fp8e4m3
    wt = nc.dram_tensor("wt", [P, KT * D], fp8, kind="ExternalInput")
    scal = nc.dram_tensor("scal", [P, 2], fp32, kind="ExternalInput")  # [scale, 1/scale]
    out = nc.dram_tensor("out", [rows, D], bf16, kind="ExternalOutput")

    with TileContext(nc) as tc:
        with (
            tc.tile_pool(name="const", bufs=1) as constp,
            tc.tile_pool(name="xin", bufs=xin_bufs) as xp,
            tc.tile_pool(name="work", bufs=work_bufs) as wp,
            tc.tile_pool(name="hlp", bufs=hl_bufs) as hlp,
            tc.tile_pool(name="ptp", bufs=pt_bufs, space="PSUM") as ptp,
            tc.tile_pool(name="pop", bufs=po_bufs, space="PSUM") as pop,
            tc.tile_pool(name="oout", bufs=out_bufs) as op_,
        ):
            ident = constp.tile([P, P], bf16)
            make_identity(nc, ident)
            sc = constp.tile([P, 2], fp32)
            nc.gpsimd.dma_start(out=sc, in_=scal[:, :])
            wt_sb = constp.tile([P, KT * D], fp8)

            state = {}

            def front(st):
                # first tiles: quarter-granular DMA + quantize so the pipe
                # fills fast; steady state: two half DMAs, one per ring
                qs_n = 4 if st < fine_tiles else 2
                Hq = D // qs_n
                xa = xp.tile([P, D], fp32, name="xa")
                t = wp.tile([P, D], fp32, name="t")
                q = wp.tile([P, D], bf16, name="q")
                qT = ptp.tile([P, D], bf16, name="qT")
                hl = hlp.tile([P, 2 * D], fp8, name="hl")
                for hq in range(qs_n):
                    hs = slice(hq * Hq, (hq + 1) * Hq)
                    eng = nc.sync if hq % 2 == 0 else nc.scalar
                    eng.dma_start(out=xa[:, hs], in_=x[st * P:(st + 1) * P, hs])
                    nc.gpsimd.tensor_scalar(t[:, hs], xa[:, hs], sc[:, 1:2], M1,
                                            Alu.mult, Alu.add)
                    nc.gpsimd.tensor_scalar(q[:, hs], t[:, hs], -M1, None, Alu.add)
                    for k in range(hq * KT // qs_n, (hq + 1) * KT // qs_n):
                        nc.tensor.transpose(
                            qT[:, k * P:(k + 1) * P], q[:, k * P:(k + 1) * P], ident)
                    nc.scalar.activation(hl[:, hq * Hq:(hq + 1) * Hq],
                                         qT[:, hs], Act.Copy)
                    nc.vector.tensor_tensor(out=hl[:, D + hq * Hq:D + (hq + 1) * Hq],
                                            in0=qT[:, hs], in1=hl[:, hq * Hq:(hq + 1) * Hq],
                                            op=Alu.subtract)
                state[st] = hl

            def back(st, last=False):
                hl = state.pop(st)
                hl3 = hl[:, :].rearrange("p (two x) -> p two x", two=2)
                oo = op_.tile([P, D], bf16, name="oo")
                po = [pop.tile([P, 512], fp32, name="po", tag="po") for _ in range(2)]
                for h in range(2):
                    for k in range(KT):
                        lhs = hl3[:, :, k * P:(k + 1) * P]
                        rhs = wt_sb[:, k * D + h * 512: k * D + h * 512 + 512]
                        rhs = rhs.unsqueeze(1).broadcast_to([P, 2, 512])
                        nc.tensor.matmul(
                            po[h], lhs, rhs,
                            start=(k == 0), stop=(k == KT - 1),
                            perf_mode=mybir.MatmulPerfMode.DoubleRow,
                        )
                    # epilogue for this half right behind its matmuls:
                    # ACT drains half 0 while PE still runs half 1
                    if h == 0:
                        nc.scalar.activation(oo[:, 0:512], po[0], Act.Copy,
                                             scale=sc[:, 0:1])
                    else:
                        nc.vector.tensor_scalar(oo[:, 512:1024], po[1],
                                                sc[:, 0:1], None, Alu.mult)
                if last:
                    # split the final store across both rings
                    nc.scalar.dma_start(out=out[st * P:(st + 1) * P, 0:512],
                                        in_=oo[:, 0:512])
                    nc.sync.dma_start(out=out[st * P:(st + 1) * P, 512:1024],
                                      in_=oo[:, 512:1024])
                else:
                    eng = nc.sync if st % 2 == 0 else nc.scalar
                    eng.dma_start(out=out[st * P:(st + 1) * P, :], in_=oo)

            # weight DMA in chunks on the scalar ring, interleaved between
            # the first two tiles' emissions (all chunks land before back(0))
            wt_cols = KT * D // wt_chunks
            for st in range(nt):
                front(st)
                if st < 2:
                    for c in range(st * wt_chunks // 2, (st + 1) * wt_chunks // 2):
                        cs = slice(c * wt_cols, (c + 1) * wt_cols)
                        nc.scalar.dma_start(out=wt_sb[:, cs], in_=wt[:, cs])
                if st >= lag:
                    back(st - lag)
            for st in range(max(nt - lag, 0), nt):
                back(st, last=(st == nt - 1))
    nc.compile()
    return nc


def _get_nc(nt=NT):
    if nt not in _NC_CACHE:
        _NC_CACHE[nt] = _build_nc(nt)
    return _NC_CACHE[nt]


def _prep_inputs(x, ternary_weight, bias, act_scale, n_cores=N_CORES, rows=ROWS):
    x = np.asarray(x, dtype=np.float32)
    tw = np.asarray(ternary_weight)

    scale = np.maximum(np.float32(act_scale), np.float32(1e-5))
    inv = np.float32(1.0) / scale

    # w.T [i, o] = tw[o, i] - 1 in {-1,0,1}, exact in fp8e4m3; fold to
    # [128, KT*D]: wt[p, k*D + o] = w[k*128+p, o]
    w = (tw.T.astype(np.float32) - 1.0).astype(F8)  # [D_IN, D_OUT]
    wt_folded = np.ascontiguousarray(
        w.reshape(KT, P, D).transpose(1, 0, 2).reshape(P, KT * D))
    scal = np.ascontiguousarray(
        np.broadcast_to(np.array([scale, inv], dtype=np.float32)[None, :], (P, 2)))

    xf = x.reshape(-1, D)
    in_maps = []
    for c in range(n_cores):
        in_maps.append({
            "x": np.ascontiguousarray(xf[c * rows:(c + 1) * rows]),
            "wt": wt_folded,
            "scal": scal,
        })
    return in_maps


def kernel(x, ternary_weight, bias, act_scale):
    from concourse.bass_utils import run_bass_kernel_spmd

    in_maps = _prep_inputs(x, ternary_weight, bias, act_scale)
    nc = _get_nc()
    res = run_bass_kernel_spmd(nc, in_maps, core_ids=list(range(N_CORES)))
    out = np.concatenate([np.asarray(r["out"]) for r in res.results], axis=0)
    out = out.astype(np.float32) + np.asarray(bias, dtype=np.float32)[None, :]
    return out.reshape(B, S, D)


# revision 20
# speedup vs baseline: 1.0247x; 1.0103x over previous
"""BitLinear forward (fake-quant int8 activations x ternary weight) on 8 TRN2 cores.

Data-parallel over the flattened (B*S) token dim: 8192 rows per core, the
[1024,1024] ternary weight replicated per core as fp8e4m3 (exact: values in
{-1,0,1}).

Per-core kernel (per 128-row tile, software-pipelined with LAG tiles between
the front and back halves):
  SP    : x tile in (fp32, 512KB); bf16 out tile alternates SP/ACT rings
  Pool  : t = x*(1/s) + M1 (fp32, magic round)    [GPSIMD tensor_scalar]
          q = t - M1 -> bf16 (exact ints in [-127,127]; no clamp needed:
          |x*(1/s)| <= 127.0001 rounds to <= 127)
  PE    : 8x 128x128 transpose of q (bf16, raw mode) -> qT in PSUM
  ACT   : h8T = Copy(qT) -> fp8e4m3.  The fp8 RNE cast IS the coarse split:
          for |q| in (2^k, 2^(k+1)], k>=4, it rounds onto a 16-level grid, so
          l = q - h8 always fits fp8 exactly (|l| <= 4).
  DVE   : lT = qT - h8T -> fp8e4m3
  PE    : 16 DoubleRow fp8 matmuls: psum[s,o] += sum_k (h8T_k.T @ w_k
          + lT_k.T @ w_k), both planes streaming the same w_k via a
          stride-0 broadcast AP.  DoubleRow = 0.5 cycles/output-column.
  ACT/DVE: epilogue halves: out = psum * scale -> bf16, each emitted right
          behind its half's matmuls.
Host: upcast bf16 -> fp32 and add bias (exact fp32 add; total error is the
bf16 output rounding, ~3.4e-3 relative, plus quantize boundary flips).

Cost model, steady state per tile: PE 2131ns (bound; 424 transpose + 1707
DoubleRow), Pool 1706, DVE ~1850, ACT ~2045 incl out-DMA queue share, SP
ring ~1975.  Full per-core pass 147.5us vs 256.3us bf16-matmul baseline.
"""

import numpy as np
import ml_dtypes

B, S, D = 16, 4096, 1024
N_CORES = 8
ROWS = (B * S) // N_CORES  # 8192 rows per core
P = 128
NT = ROWS // P             # 64 row tiles per core
KT = D // P                # 8 contraction tiles
QB = 127.0
M1 = float(1.5 * 2 ** 23)  # fp32 round-to-nearest-even magic constant
F8 = ml_dtypes.float8_e4m3

_NC_CACHE = {}


def _build_nc(nt=NT, lag=2, xin_bufs=5, work_bufs=5, hl_bufs=5, out_bufs=4,
              pt_bufs=4, po_bufs=4, wt_chunks=4, fine_tiles=4, ep_split=512):
    import concourse.mybir as mybir
    from concourse import bacc
    from concourse.tile import TileContext
    from concourse.masks import make_identity

    fp32 = mybir.dt.float32
    bf16 = mybir.dt.bfloat16
    fp8 = mybir.dt.float8e4
    Alu = mybir.AluOpType
    Act = mybir.ActivationFunctionType

    nc = bacc.Bacc(None, target_bir_lowering=False)
    rows = nt * P
    x = nc.dram_tensor("x", [rows, D], fp32, kind="ExternalInput")
    # wt[p, k*D + o] = w[k*128+p, o], w = ternary_weight.T - 1, fp8e4m3
    wt = nc.dram_tensor("wt", [P, KT * D], fp8, kind="ExternalInput")
    scal = nc.dram_tensor("scal", [P, 2], fp32, kind="ExternalInput")  # [scale, 1/scale]
    out = nc.dram_tensor("out", [rows, D], bf16, kind="ExternalOutput")

    with TileContext(nc) as tc:
        with (
            tc.tile_pool(name="const", bufs=1) as constp,
            tc.tile_pool(name="xin", bufs=xin_bufs) as xp,
            tc.tile_pool(name="work", bufs=work_bufs) as wp,
            tc.tile_pool(name="hlp", bufs=hl_bufs) as hlp,
            tc.tile_pool(name="ptp", bufs=pt_bufs, space="PSUM") as ptp,
            tc.tile_pool(name="pop", bufs=po_bufs, space="PSUM") as pop,
            tc.tile_pool(name="oout", bufs=out_bufs) as op_,
        ):
            ident = constp.tile([P, P], bf16)
            make_identity(nc, ident)
            sc = constp.tile([P, 2], fp32)
            nc.gpsimd.dma_start(out=sc, in_=scal[:, :])
            wt_sb = constp.tile([P, KT * D], fp8)
            state = {}

            def front(st):
                # first tiles run quarter-granular DMA + compute so the
                # pipe fills fast; steady state is one full-width pass
                qs_n = 4 if st < fine_tiles else 1
                Hq = D // qs_n
                xa = xp.tile([P, D], fp32, name="xa")
                t = wp.tile([P, D], fp32, name="t")
                q = wp.tile([P, D], bf16, name="q")
                qT = ptp.tile([P, D], bf16, name="qT")
                hl = hlp.tile([P, 2 * D], fp8, name="hl")
                dma_n = qs_n if qs_n > 1 else 1
                Hd = D // dma_n
                for hd in range(dma_n):
                    hs = slice(hd * Hd, (hd + 1) * Hd)
                    nc.sync.dma_start(out=xa[:, hs], in_=x[st * P:(st + 1) * P, hs])
                for hq in range(qs_n):
                    hs = slice(hq * Hq, (hq + 1) * Hq)
                    nc.gpsimd.tensor_scalar(t[:, hs], xa[:, hs], sc[:, 1:2], M1,
                                            Alu.mult, Alu.add)
                    nc.gpsimd.tensor_scalar(q[:, hs], t[:, hs], -M1, None,
                                            Alu.add)
                    for k in range(hq * KT // qs_n, (hq + 1) * KT // qs_n):
                        nc.tensor.transpose(
                            qT[:, k * P:(k + 1) * P], q[:, k * P:(k + 1) * P], ident)
                    nc.scalar.activation(hl[:, hq * Hq:(hq + 1) * Hq],
                                         qT[:, hs], Act.Copy)
                    nc.vector.tensor_tensor(out=hl[:, D + hq * Hq:D + (hq + 1) * Hq],
                                            in0=qT[:, hs], in1=hl[:, hq * Hq:(hq + 1) * Hq],
                                            op=Alu.subtract)
                state[st] = hl

            def back(st, last=False):
                hl = state.pop(st)
                hl3 = hl[:, :].rearrange("p (two x) -> p two x", two=2)
                oo = op_.tile([P, D], bf16, name="oo")
                po = [pop.tile([P, 512], fp32, name="po", tag="po") for _ in range(2)]
                for h in range(2):
                    for k in range(KT):
                        lhs = hl3[:, :, k * P:(k + 1) * P]
                        rhs = wt_sb[:, k * D + h * 512: k * D + h * 512 + 512]
                        rhs = rhs.unsqueeze(1).broadcast_to([P, 2, 512])
                        nc.tensor.matmul(
                            po[h], lhs, rhs,
                            start=(k == 0), stop=(k == KT - 1),
                            perf_mode=mybir.MatmulPerfMode.DoubleRow,
                        )
                    # epilogue right behind each half's matmuls; ACT takes
                    # [0:ep_split], DVE the rest (load-balance the engines)
                    if h == 0:
                        eps0 = min(ep_split, 512)
                        nc.scalar.activation(oo[:, 0:eps0],
                                             po[0][:, 0:eps0], Act.Copy,
                                             scale=sc[:, 0:1])
                        if ep_split < 512:
                            nc.vector.tensor_scalar(oo[:, ep_split:512],
                                                    po[0][:, ep_split:512],
                                                    sc[:, 0:1], None, Alu.mult)
                    else:
                        if ep_split > 512:
                            nc.scalar.activation(oo[:, 512:ep_split],
                                                 po[1][:, 0:ep_split - 512],
                                                 Act.Copy, scale=sc[:, 0:1])
                        nc.vector.tensor_scalar(oo[:, max(512, ep_split):1024],
                                                po[1][:, max(0, ep_split - 512):512],
                                                sc[:, 0:1], None, Alu.mult)
                if last:
                    # split the final store across both rings
                    nc.scalar.dma_start(out=out[st * P:(st + 1) * P, 0:512],
                                        in_=oo[:, 0:512])
                    nc.sync.dma_start(out=out[st * P:(st + 1) * P, 512:1024],
                                      in_=oo[:, 512:1024])
                else:
                    eng = nc.sync if st % 2 == 0 else nc.scalar
                    eng.dma_start(out=out[st * P:(st + 1) * P, :], in_=oo)

            # weight DMA in chunks on the scalar ring, interleaved between
            # the first two tiles' emissions (all chunks land before back(0))
            wt_cols = KT * D // wt_chunks
            for st in range(nt):
                front(st)
                if st < 2:
                    for c in range(st * wt_chunks // 2, (st + 1) * wt_chunks // 2):
                        cs = slice(c * wt_cols, (c + 1) * wt_cols)
                        nc.scalar.dma_start(out=wt_sb[:, cs], in_=wt[:, cs])
                if st >= lag:
                    back(st - lag)
            for st in range(max(nt - lag, 0), nt):
                back(st, last=(st == nt - 1))
    nc.compile()
    return nc


def _get_nc(nt=NT):
    if nt not in _NC_CACHE:
        _NC_CACHE[nt] = _build_nc(nt)
    return _NC_CACHE[nt]


def _prep_inputs(x, ternary_weight, bias, act_scale, n_cores=N_CORES, rows=ROWS):
    x = np.asarray(x, dtype=np.float32)
    tw = np.asarray(ternary_weight)

    scale = np.maximum(np.float32(act_scale), np.float32(1e-5))
    inv = np.float32(1.0) / scale

    # w.T [i, o] = tw[o, i] - 1 in {-1,0,1}, exact in fp8e4m3; fold to
    # [128, KT*D]: wt[p, k*D + o] = w[k*128+p, o]
    w = (tw.T.astype(np.float32) - 1.0).astype(F8)  # [D_IN, D_OUT]
    wt_folded = np.ascontiguousarray(
        w.reshape(KT, P, D).transpose(1, 0, 2).reshape(P, KT * D))
    scal = np.ascontiguousarray(
        np.broadcast_to(np.array([scale, inv], dtype=np.float32)[None, :], (P, 2)))

    xf = x.reshape(-1, D)
    in_maps = []
    for c in range(n_cores):
        in_maps.append({
            "x": np.ascontiguousarray(xf[c * rows:(c + 1) * rows]),
            "wt": wt_folded,
            "scal": scal,
        })
    return in_maps


def kernel(x, ternary_weight, bias, act_scale):
    from concourse.bass_utils import run_bass_kernel_spmd

    in_maps = _prep_inputs(x, ternary_weight, bias, act_scale)
    nc = _get_nc()
    res = run_bass_kernel_spmd(nc, in_maps, core_ids=list(range(N_CORES)))
    out = np.concatenate([np.asarray(r["out"]) for r in res.results], axis=0)
    out = out.astype(np.float32) + np.asarray(bias, dtype=np.float32)[None, :]
    return out.reshape(B, S, D)
